# revision 6
# baseline (speedup 1.0000x reference)
"""TRN2 Bass kernel for nn_MetrixSoftmax: softmax(-2 * ||x_b - w_o||_2, axis=o).

x: [8192, 256] f32, weight: [16384, 256] f32 -> out: [8192, 16384] f32.

Strategy: data-parallel shard x over batch across 8 cores (1024 rows each),
replicate weight; each core computes its full output rows so the softmax
needs no collectives. Per core (layout: partitions=batch rows, free=out):

  d2[b,o] = (x2[b]+256) + (w2[o]-256) - 2*x.w
  psum    = matmul-accumulated [-2*x.w + (w2-256)]  (w2 row folded in as a
            K=3 bf16 matmul of an exact 3-way bf16 split of w2-256)
  dist    = ACT Sqrt(psum + bias[b])   (bias = x2+256, per-partition AP)
  e       = ACT Exp(-2*dist), accum_out -> per-row partial sums
  out     = e * (1/sum)                (DVE reciprocal + tensor_scalar_mul)

Matmul modes:
  split7: x and w.T each split hi/lo bf16; 6 bf16 matmuls (hh, hl, lh) +
          w2 row -> ~7e-4 abs err on d2 (fp32-grade output).
  f32r3:  x, w.T fed as raw fp32 bits declared float32r (tf32-like 1-pass);
          2 matmuls + w2 row -> ~4e-2 abs err on d2 (~1e-3 output rel err).

Performance structure: chunks of G=4 psum banks [128, 2048]; matmuls run
product-major inside a group so the PE stationary operand is reused 4x;
ACT processes 2048-wide chunks; sqrt/exp phases are strictly ordered per
batch-tile to get exactly 2 ACT table loads per tile.
"""

import numpy as np
import ml_dtypes

B, IN, OUT = 8192, 256, 16384
NCORES = 8
BPC = B // NCORES     # 1024 batch rows per core
NT = BPC // 128       # 8 batch tiles of 128 rows
CH = 512              # matmul free-dim (one PSUM bank)
GRP = 4               # chunks per psum/slab group
GW = CH * GRP         # 2048 group width
NG = OUT // GW        # 8 groups per batch tile

MODE = "split7"       # "split7" (accurate) | "f32r3" (fast)

_BF16 = ml_dtypes.bfloat16
_built = {}


def _bf16_split(a):
    hi = a.astype(_BF16)
    lo = (a - hi.astype(np.float32)).astype(_BF16)
    return hi, lo


def _build(mode):
    import concourse.bacc as bacc
    import concourse.tile as tile
    import concourse.mybir as mybir
    from concourse.tile import add_dep_helper

    F32 = mybir.dt.float32
    F32R = mybir.dt.float32r
    BF16 = mybir.dt.bfloat16
    AF = mybir.ActivationFunctionType

    nc = bacc.Bacc("TRN2", target_bir_lowering=False, debug=False,
                   num_devices=NCORES)

    if mode == "split7":
        d_wh = nc.dram_tensor("wh", [IN, OUT], BF16, kind="ExternalInput")
        d_wl = nc.dram_tensor("wl", [IN, OUT], BF16, kind="ExternalInput")
        d_xh = nc.dram_tensor("xh", [IN, BPC], BF16, kind="ExternalInput")
        d_xl = nc.dram_tensor("xl", [IN, BPC], BF16, kind="ExternalInput")
    else:
        d_wt = nc.dram_tensor("wt", [IN, OUT], F32R, kind="ExternalInput")
        d_xt = nc.dram_tensor("xt", [IN, BPC], F32R, kind="ExternalInput")
    d_w2s = nc.dram_tensor("w2s", [3, OUT], BF16, kind="ExternalInput")
    d_x2b = nc.dram_tensor("x2b", [128, NT], F32, kind="ExternalInput")
    d_out = nc.dram_tensor("out", [BPC, OUT], F32, kind="ExternalOutput")

    from contextlib import ExitStack
    with tile.TileContext(nc) as tc, ExitStack() as ctx:
        persist = ctx.enter_context(tc.tile_pool(name="persist", bufs=1))
        xt_pool = ctx.enter_context(tc.tile_pool(name="xtp", bufs=2))
        slab_pool = ctx.enter_context(tc.tile_pool(name="slabp", bufs=NG + 1))
        w2_pool = ctx.enter_context(tc.tile_pool(name="w2p", bufs=1))
        sums_pool = ctx.enter_context(tc.tile_pool(name="sumsp", bufs=2))
        psum_pool = ctx.enter_context(tc.tile_pool(name="psump", bufs=2, space="PSUM"))

        # ---- preload weights (split column-wise so early matmuls start early)
        if mode == "split7":
            wh0 = persist.tile([128, OUT], BF16, name="wh0")
            wh1 = persist.tile([128, OUT], BF16, name="wh1")
            wl0 = persist.tile([128, OUT], BF16, name="wl0")
            wl1 = persist.tile([128, OUT], BF16, name="wl1")
            wparts = [(wh0, d_wh, 0), (wh1, d_wh, 128), (wl0, d_wl, 0), (wl1, d_wl, 128)]
        else:
            wr0 = persist.tile([128, OUT], F32R, name="wr0")
            wr1 = persist.tile([128, OUT], F32R, name="wr1")
            wparts = [(wr0, d_wt, 0), (wr1, d_wt, 128)]
        NSPLIT = 8
        CW = OUT // NSPLIT
        for j in range(NSPLIT):
            cs = slice(j * CW, (j + 1) * CW)
            for t_sb, t_dram, p0 in wparts:
                nc.sync.dma_start(t_sb[:, cs], t_dram[p0:p0 + 128, cs])

        x2sb = persist.tile([128, NT], F32, name="x2sb")
        nc.sync.dma_start(x2sb[:], d_x2b[:, :])
        ones3 = persist.tile([3, 128], BF16, name="ones3")
        nc.vector.memset(ones3[:], 1.0)

        prev_exp_insts = None
        for t in range(NT):
            ts = slice(t * 128, (t + 1) * 128)
            bias_ap = x2sb[:, t:t + 1]

            # stationary x slices for this batch tile
            if mode == "split7":
                xh0t = xt_pool.tile([128, 128], BF16, name=f"xh0t_{t}", tag="xh0t")
                xh1t = xt_pool.tile([128, 128], BF16, name=f"xh1t_{t}", tag="xh1t")
                xl0t = xt_pool.tile([128, 128], BF16, name=f"xl0t_{t}", tag="xl0t")
                xl1t = xt_pool.tile([128, 128], BF16, name=f"xl1t_{t}", tag="xl1t")
                nc.sync.dma_start(xh0t[:], d_xh[0:128, ts])
                nc.sync.dma_start(xh1t[:], d_xh[128:256, ts])
                nc.sync.dma_start(xl0t[:], d_xl[0:128, ts])
                nc.sync.dma_start(xl1t[:], d_xl[128:256, ts])
                # (stationary, moving-tensor, k-half) product list; grouped so
                # consecutive matmuls share the stationary where possible
                products = [(xh0t, wh0), (xh0t, wl0), (xl0t, wh0),
                            (xh1t, wh1), (xh1t, wl1), (xl1t, wh1)]
            else:
                xr0t = xt_pool.tile([128, 128], F32R, name=f"xr0t_{t}", tag="xr0t")
                xr1t = xt_pool.tile([128, 128], F32R, name=f"xr1t_{t}", tag="xr1t")
                nc.sync.dma_start(xr0t[:], d_xt[0:128, ts])
                nc.sync.dma_start(xr1t[:], d_xt[128:256, ts])
                products = [(xr0t, wr0), (xr1t, wr1)]
            nprod = len(products) + 1  # + w2 row

            slabs = []
            sqrt_insts = []
            sums = sums_pool.tile([128, NG], F32, name=f"sums_{t}", tag="sums")
            # ---- phase S: matmul groups + 2048-wide sqrt
            for g in range(NG):
                gs = slice(g * GW, (g + 1) * GW)
                w2t = w2_pool.tile([3, GW], BF16, name=f"w2t_{t}_{g}", tag="w2t")
                nc.gpsimd.dma_start(w2t[:], d_w2s[:, gs])
                ps = psum_pool.tile([128, GW], F32, name=f"ps_{t}_{g}", tag="ps")
                # product-major: stationary reused across the GRP sub-chunks
                for p, (stat, mov) in enumerate(products):
                    for i in range(GRP):
                        cs = slice(g * GW + i * CH, g * GW + (i + 1) * CH)
                        nc.tensor.matmul(ps[:, i * CH:(i + 1) * CH],
                                         stat[:], mov[:, cs],
                                         start=(p == 0), stop=False)
                for i in range(GRP):
                    nc.tensor.matmul(ps[:, i * CH:(i + 1) * CH],
                                     ones3[:, :], w2t[:, i * CH:(i + 1) * CH],
                                     start=False, stop=True)
                sl = slab_pool.tile([128, GW], F32, name=f"slab_{t}_{g}", tag="slab")
                # DVE drains psum (and adds the x2 bias) so ACT never gates
                # the PE's psum recycling; slab pool is the PE runway.
                nc.vector.tensor_scalar_add(sl[:], ps[:], bias_ap)
                sq = nc.scalar.activation(sl[:], sl[:], AF.Sqrt)
                slabs.append(sl)
                sqrt_insts.append(sq)
                if prev_exp_insts is not None:
                    add_dep_helper(sq.ins, prev_exp_insts[-1].ins,
                                   reason="ACT phase order: sqrt after prev tile exps")
            # ---- phase E: 2048-wide exp with accumulated row sums
            exp_insts = []
            for g in range(NG):
                ex = nc.scalar.activation(slabs[g][:], slabs[g][:], AF.Exp,
                                          scale=-2.0, accum_out=sums[:, g:g + 1])
                add_dep_helper(ex.ins, sqrt_insts[-1].ins,
                               reason="ACT phase order: exp after all sqrts in tile")
                exp_insts.append(ex)
            prev_exp_insts = exp_insts
            # ---- normalize + store
            tot = sums_pool.tile([128, 1], F32, name=f"tot_{t}", tag="tot")
            nc.vector.reduce_sum(tot[:], sums[:], axis=mybir.AxisListType.X)
            rec = sums_pool.tile([128, 1], F32, name=f"rec_{t}", tag="rec")
            nc.vector.reciprocal(rec[:], tot[:])
            for g in range(NG):
                gs = slice(g * GW, (g + 1) * GW)
                nc.vector.tensor_scalar_mul(slabs[g][:], slabs[g][:], rec[:, 0:1])
                nc.sync.dma_start(d_out[ts, gs], slabs[g][:])

    nc.compile()
    return nc


def _get_nc(mode):
    if mode not in _built:
        _built[mode] = _build(mode)
    return _built[mode]


def _prep_inputs(x, weight, mode):
    x = np.ascontiguousarray(np.asarray(x, dtype=np.float32))
    weight = np.ascontiguousarray(np.asarray(weight, dtype=np.float32))
    assert x.shape == (B, IN) and weight.shape == (OUT, IN)

    wt = np.ascontiguousarray(weight.T).astype(np.float32)       # [IN, OUT]
    w2 = np.sum(weight.astype(np.float64) ** 2, axis=1)
    w2c = (w2 - 256.0).astype(np.float32)
    w2a = w2c.astype(_BF16)
    r1 = w2c - w2a.astype(np.float32)
    w2b = r1.astype(_BF16)
    w2d = (r1 - w2b.astype(np.float32)).astype(_BF16)
    w2s = np.ascontiguousarray(np.stack([w2a, w2b, w2d], axis=0))  # [3, OUT]

    shared = {"w2s": w2s}
    if mode == "split7":
        wh, wl = _bf16_split(wt)
        shared["wh"] = wh
        shared["wl"] = wl
    else:
        shared["wt"] = wt  # raw fp32 bits, declared float32r on device

    in_maps = []
    for i in range(NCORES):
        xs = x[i * BPC:(i + 1) * BPC]                             # [BPC, IN]
        xt = np.ascontiguousarray((-2.0 * xs.T).astype(np.float32))  # [IN, BPC]
        x2 = np.sum(xs.astype(np.float64) ** 2, axis=1).astype(np.float32) + 256.0
        x2b = np.ascontiguousarray(x2.reshape(NT, 128).T).astype(np.float32)
        m = dict(shared)
        if mode == "split7":
            xh, xl = _bf16_split(xt)
            m["xh"] = xh
            m["xl"] = xl
        else:
            m["xt"] = xt
        m["x2b"] = x2b
        in_maps.append(m)
    return in_maps


def _run(x, weight, mode=None, trace=False, trace_cores=None):
    from concourse.bass_utils import run_bass_kernel_spmd
    mode = mode or MODE
    nc = _get_nc(mode)
    in_maps = _prep_inputs(x, weight, mode)
    res = run_bass_kernel_spmd(nc, in_maps, list(range(NCORES)), trace=trace,
                               trace_cores=trace_cores)
    out = np.concatenate([res.results[i]["out"] for i in range(NCORES)], axis=0)
    return out, res


def kernel(x, weight):
    out, _ = _run(x, weight)
    return out


def kernel_profiled(x, weight, mode=None, trace_cores=None):
    """Returns (out, exec_time_ns, trace_path)."""
    out, res = _run(x, weight, mode=mode, trace=True, trace_cores=trace_cores)
    trace_path = None
    if res.instructions_and_trace is not None:
        trace_path = res.instructions_and_trace[1]
    return out, res.exec_time_ns, trace_path
